# revision 37
# baseline (speedup 1.0000x reference)
"""MoE (top-2, 8 experts) SwiGLU kernel for 8 Trainium2 NeuronCores.

Strategy (expert-parallel, per the sharding hint):
  - Host: router matmul + top-2 + softmax (tiny: [4096,1024]@[1024,8]),
    build per-expert token permutation (token-major order, capacity-truncated
    exactly like the reference's jnp.nonzero(size=CAPACITY)).
  - Device (core e == expert e): fused SwiGLU over the FIRST C_DEV=1024
    tokens of expert e (the perfectly-balanced per-core share):
        hT = (W_e^T x^T) * silu(V_e^T x^T)        [H, C] layout
        y  = (hT)^T-contract @ Wout_e             [D, C] layout, bf16 out
    All matmuls bf16 with fp32 PSUM accumulation; weights resident in SBUF.
  - Host: the few overflow tokens (expert load above C_DEV, ~1% of work,
    pure load imbalance that SPMD cannot express) are computed in fp32
    numpy; then inverse-permutation gather + prob scaling + top-2 sum.

Timing notes (from NTFF/perfetto analysis of prior runs): the PE streams
matmuls back-to-back at 1 col/cycle (2.4 GHz); exec time = fixed prologue
(~8.6us) + head DMA delivery (~1.5 MB gates the first hb group) + PE work
(384 cy/token) + flush + teardown. Levers used here:
  - C_DEV = 1024 fixed: PE work is the balanced minimum; overflow to host.
  - the whole first-matmul dependency (chunk-0 x ++ slab-0 W|V) is ONE
    host-packed "hx" tensor delivered by a single large DMA (~340 GB/s vs
    ~180 for small pieces); later W|V slabs are one dma_start each (the
    Sync engine's ~600 ns/dma_start issue rate is a real constraint, and
    the second HWDGE ring must stay unused - it throttles the PE ~20%).
  - warm-up matmuls on a zeroed tile raise the PE p-state (0.65->2.4 GHz
    ramp) during the head DMA wait; slight overshoot is preferred since an
    idle gap drops the p-state again (~3 us penalty).
  - hT double-buffered in two half-tiles so phase B's accumulation starts
    right after hb=7's multiply instead of hb=15's.
  - bf16 y + a 256-col final chunk keep the output flush short.
"""

import numpy as np
import ml_dtypes

import concourse.bass as bass  # noqa: F401  (bass types referenced via bacc/tile)
import concourse.mybir as mybir
import concourse.tile as tile
from concourse import bacc
from concourse.bass_utils import run_bass_kernel_spmd

B, T = 2, 2048
D_MODEL, D_HIDDEN = 1024, 2048
N_EXPERTS, TOP_K = 8, 2
N_TOKENS = B * T
CAPACITY = 2 * N_TOKENS * TOP_K // N_EXPERTS  # 2048
C_DEV = N_TOKENS * TOP_K // N_EXPERTS         # 1024: balanced per-core share

F32 = mybir.dt.float32
BF16 = mybir.dt.bfloat16
AF = mybir.ActivationFunctionType
BF = ml_dtypes.bfloat16

N_WARMUP = 20  # junk matmuls covering the head DMA wait (p-state ramp)

_KERNEL_CACHE: dict = {}


def _chunk_schedule(C: int, width: int = 384):
    """Full `width` chunks; the ragged tail keeps >=192 cols (tiny chunks
    are latency-bound: a 47-col chunk measured ~2x its streaming time)."""
    chunks = []
    c0 = 0
    while C - c0 > width + 128:
        chunks.append((c0, width))
        c0 += width
    chunks.append((c0, C - c0))
    assert chunks[-1][1] <= 512  # one PSUM bank
    return chunks


def _build_expert_kernel(C: int, D: int = D_MODEL, H: int = D_HIDDEN):
    assert D % 128 == 0 and H % 128 == 0 and C % 128 == 0
    DK, HB = D // 128, H // 128
    nc = bacc.Bacc(None, target_bir_lowering=False, debug=False)

    xT = nc.dram_tensor("xT", [128, DK, C], BF16, kind="ExternalInput")
    # V and W interleaved per hb-slab ([..., :128]=V, [..., 128:]=W) so each
    # slab is ONE dma_start: the Sync engine's descriptor issue rate
    # (~600 ns per dma_start) is a real head-latency constraint.
    WVd = nc.dram_tensor("WV", [HB, 128, DK, 256], BF16, kind="ExternalInput")
    chunks = _chunk_schedule(C)
    cols_0 = chunks[0][1]
    chunk = max(c for _, c in chunks)
    HBH = HB // 2
    # hx = chunk-0 x ++ slab-0 WV, host-packed into ONE tensor: the whole
    # first-matmul dependency arrives as a single large DMA (~341 GB/s)
    # instead of several small ones (~180 GB/s).
    hxd = nc.dram_tensor("hx", [128, DK, cols_0 + 256], BF16,
                         kind="ExternalInput")
    Wo = nc.dram_tensor("Wo", [H, D], BF16, kind="ExternalInput")
    # y is produced TRANSPOSED [D, C]: tokens on the matmul free dim.
    y = nc.dram_tensor("y", [D, C], BF16, kind="ExternalOutput")

    with tile.TileContext(nc) as tc:
        with (
            tc.tile_pool(name="wpool", bufs=1) as wpool,
            tc.tile_pool(name="hpool", bufs=2) as hpool,
            tc.tile_pool(name="spool", bufs=3) as spool,
            tc.tile_pool(name="ypool", bufs=3) as ypool,
            tc.tile_pool(name="pa", bufs=2, space="PSUM") as pa_pool,
            tc.tile_pool(name="pb", bufs=2, space="PSUM") as pb_pool,
            tc.tile_pool(name="py", bufs=2, space="PSUM") as py_pool,
            tc.tile_pool(name="pw", bufs=1, space="PSUM") as pw_pool,
        ):
            hx_sb = wpool.tile([128, DK, cols_0 + 256], BF16, tag="hx")
            x_tiles = [None] + [
                wpool.tile([128, DK, cols], BF16, tag=f"x{i}", name=f"x{i}")
                for i, (_, cols) in enumerate(chunks) if i > 0]
            WV_tiles = [None] + [
                wpool.tile([128, DK, 256], BF16, tag=f"WV{hb}",
                           name=f"WVt{hb}") for hb in range(1, HB)]
            # Wo in two column halves so phase B's later nb's don't gate on
            # one monolithic 4.2 MB transfer.
            Wo_lo = wpool.tile([128, HB, D // 2], BF16, tag="Wo_lo")
            Wo_hi = wpool.tile([128, HB, D // 2], BF16, tag="Wo_hi")
            warm = wpool.tile([128, chunk], BF16, tag="warm")

            xT_r = xT[:]
            Wo_r = Wo[:].rearrange("(b p) d -> p b d", p=128)

            # PE warm-up: junk matmuls on a zeroed tile raise the tensor
            # engine out of its low p-state while the head DMAs land.
            nc.vector.memset(warm[:], 0.0)
            pwarm = pw_pool.tile([128, chunk], F32, tag="pw")
            for _ in range(N_WARMUP):
                nc.tensor.matmul(pwarm[:], warm[:, :128], warm[:],
                                 start=True, stop=True)

            # DMA issue order ~= need order. Few, large transfers on the SP
            # ring only: each dma_start costs ~600 ns of Sync issue time that
            # delays every later transfer, and using the second (Activation)
            # HWDGE ring measurably throttles the PE (ordering mode) — a
            # ~20% uniform matmul slowdown, so it must stay unused.
            nc.sync.dma_start(out=hx_sb[:], in_=hxd[:])
            for hb in range(1, HB):
                nc.sync.dma_start(out=WV_tiles[hb][:], in_=WVd[hb])
            nc.sync.dma_start(out=Wo_lo[:], in_=Wo_r[:, :, :D // 2])
            if len(chunks) > 1:
                nc.sync.dma_start(out=x_tiles[1][:],
                                  in_=xT_r[:, :, chunks[1][0]:
                                           chunks[1][0] + chunks[1][1]])
            nc.sync.dma_start(out=Wo_hi[:], in_=Wo_r[:, :, D // 2:])
            for i, (c0, cols) in enumerate(chunks):
                if i <= 1:
                    continue
                nc.sync.dma_start(out=x_tiles[i][:],
                                  in_=xT_r[:, :, c0:c0 + cols])

            y_r = y[:].rearrange("(m p) c -> p m c", p=128)

            for i, (c0, cols) in enumerate(chunks):
                # hT in two half-tiles: phase B's first accumulation steps
                # (hb 0..7) only wait for the lower half's multiplies.
                hT_lo = hpool.tile([128, HBH, chunk], BF16, tag="hTl")
                hT_hi = hpool.tile([128, HBH, chunk], BF16, tag="hTh")
                for hb in range(HB):
                    hT = hT_lo if hb < HBH else hT_hi
                    hj = hb if hb < HBH else hb - HBH
                    pa = pa_pool.tile([128, chunk], F32, tag="pa")
                    pb = pb_pool.tile([128, chunk], F32, tag="pb")

                    def lhs(dk, off):
                        if hb == 0:
                            return hx_sb[:, dk, cols_0 + off:cols_0 + off + 128]
                        return WV_tiles[hb][:, dk, off:off + 128]

                    def xin(dk):
                        if i == 0:
                            return hx_sb[:, dk, :cols]
                        return x_tiles[i][:, dk, :cols]

                    for dk in range(DK):
                        nc.tensor.matmul(
                            pb[:, :cols], lhs(dk, 0), xin(dk),
                            start=(dk == 0), stop=(dk == DK - 1),
                        )
                    for dk in range(DK):
                        nc.tensor.matmul(
                            pa[:, :cols], lhs(dk, 128), xin(dk),
                            start=(dk == 0), stop=(dk == DK - 1),
                        )
                    sg = spool.tile([128, chunk], F32, tag="sg")
                    nc.scalar.activation(sg[:, :cols], pb[:, :cols], AF.Silu)
                    nc.vector.tensor_mul(hT[:, hj, :cols], pa[:, :cols],
                                         sg[:, :cols])
                for nb in range(D // 128):
                    Wo_sb = Wo_lo if nb < D // 256 else Wo_hi
                    nj = nb * 128 if nb < D // 256 else nb * 128 - D // 2
                    py = py_pool.tile([128, chunk], F32, tag="py")
                    for hb in range(HB):
                        hT = hT_lo if hb < HBH else hT_hi
                        hj = hb if hb < HBH else hb - HBH
                        nc.tensor.matmul(
                            py[:, :cols],
                            Wo_sb[:, hb, nj:nj + 128],
                            hT[:, hj, :cols],
                            start=(hb == 0), stop=(hb == HB - 1),
                        )
                    ysb = ypool.tile([128, chunk], BF16, tag="y")
                    # copy on DVE, not Scalar: 2x rate for 16-bit out and
                    # idle at the tail, so the final flush starts ~0.2us
                    # sooner (PSUM reads from DVE are fine - phase A does it)
                    nc.vector.tensor_scalar_mul(ysb[:, :cols], py[:, :cols],
                                                1.0)
                    nc.sync.dma_start(out=y_r[:, nb, c0:c0 + cols],
                                      in_=ysb[:, :cols])
    nc.compile()
    return nc


def _get_kernel(C: int, D: int = D_MODEL, H: int = D_HIDDEN):
    key = (C, D, H)
    nc = _KERNEL_CACHE.get(key)
    if nc is None:
        nc = _build_expert_kernel(C, D, H)
        _KERNEL_CACHE[key] = nc
    return nc


def _router_logits(x_flat: np.ndarray, router_w: np.ndarray,
                   router_b: np.ndarray) -> np.ndarray:
    # Prefer jax-on-CPU so near-tie top-k decisions match the reference's
    # fp32 rounding as closely as possible; fall back to numpy.
    try:
        import jax
        import jax.numpy as jnp
        cpu = jax.devices("cpu")[0]
        with jax.default_device(cpu):
            lg = jnp.asarray(x_flat) @ jnp.asarray(router_w).T + jnp.asarray(router_b)
            return np.asarray(jax.device_get(lg)).astype(np.float32, copy=False)
    except Exception:
        return (x_flat @ router_w.T + router_b).astype(np.float32)


def kernel(x, router_w, router_b, W, V, W_out):
    Bq, Tq, D = x.shape
    N = Bq * Tq
    x_flat = np.ascontiguousarray(x, dtype=np.float32).reshape(N, D)

    # ---- routing (host) ----
    logits = _router_logits(x_flat, router_w, router_b)          # [N, E]
    order2 = np.argsort(-logits, axis=1, kind="stable")[:, :TOP_K]  # lax.top_k ties
    top_ids = order2.astype(np.int64)                            # [N, K]
    top_vals = np.take_along_axis(logits, top_ids, axis=1)
    mx = top_vals.max(axis=1, keepdims=True)
    ex = np.exp((top_vals - mx).astype(np.float32))
    probs = (ex / ex.sum(axis=1, keepdims=True)).astype(np.float32)

    # ---- permutation (token-major scan order, capacity truncation) ----
    flat_e = top_ids.ravel()                                     # [N*K]
    scan = np.argsort(flat_e, kind="stable")                     # grouped by expert
    counts = np.bincount(flat_e, minlength=N_EXPERTS)
    starts = np.zeros(N_EXPERTS + 1, dtype=np.int64)
    starts[1:] = np.cumsum(counts)
    C = C_DEV                                     # fixed device width

    x_pad = np.vstack([x_flat, np.zeros((1, D), np.float32)])
    probs_pad = np.vstack([probs, np.zeros((1, TOP_K), np.float32)])

    tok_pad = np.full((N_EXPERTS, C), N, dtype=np.int64)
    slot_pad = np.zeros((N_EXPERTS, C), dtype=np.int64)
    pos_of_pair = np.full(N * TOP_K, -1, dtype=np.int64)
    ov_tok, ov_slot, ov_expert = [], [], []       # load-imbalance overflow
    for e in range(N_EXPERTS):
        idxs = scan[starts[e]:starts[e + 1]][:CAPACITY]
        dev, ov = idxs[:C], idxs[C:]
        tok_pad[e, :len(dev)] = dev // TOP_K
        slot_pad[e, :len(dev)] = dev % TOP_K
        pos_of_pair[dev] = e * C + np.arange(len(dev))
        if len(ov):
            pos_of_pair[ov] = N_EXPERTS * C + len(ov_tok) + np.arange(len(ov))
            ov_tok.extend(ov // TOP_K)
            ov_slot.extend(ov % TOP_K)
            ov_expert.extend([e] * len(ov))

    # ---- per-core device inputs ----
    def _pack(mat):  # [D, H] -> [HB, 128, DK, 128] partition-major slabs
        Dm, Hm = mat.shape
        return np.ascontiguousarray(
            mat.astype(BF).reshape(Dm // 128, 128, Hm // 128, 128)
            .transpose(2, 1, 0, 3))

    cols_0 = _chunk_schedule(C)[0][1]
    in_maps = []
    w_scales = []
    for e in range(N_EXPERTS):
        xg = x_pad[tok_pad[e]]                                   # [C, D]
        w_e = probs_pad[tok_pad[e], slot_pad[e]].astype(np.float32)  # [C]
        xTp = (xg.T.astype(BF)                                   # [D, C] ->
               .reshape(D // 128, 128, C).transpose(1, 0, 2))    # [128, DK, C]
        WVp = np.concatenate([_pack(V[e]), _pack(W[e])], axis=-1)
        hxp = np.concatenate([xTp[:, :, :cols_0], WVp[0]], axis=-1)
        in_maps.append({
            "xT": np.ascontiguousarray(xTp),
            "WV": np.ascontiguousarray(WVp),
            "hx": np.ascontiguousarray(hxp),
            "Wo": W_out[e].astype(BF),
        })
        w_scales.append(w_e)

    # ---- run on 8 cores ----
    H = W.shape[2]
    nc = _get_kernel(C, D, H)
    res = None
    for attempt in range(2):
        try:
            res = run_bass_kernel_spmd(nc, in_maps,
                                       core_ids=list(range(N_EXPERTS)))
            break
        except Exception as err:  # transient axon/device errors: retry once
            import sys
            print(f"kernel: device run attempt {attempt} failed: {err!r}",
                  file=sys.stderr)
    if res is not None:
        y_list = [np.asarray(res.results[e]["y"], dtype=np.float32).T
                  * w_scales[e][:, None]
                  for e in range(N_EXPERTS)]
    else:  # last resort so a flaky device doesn't turn into a crash
        import sys
        print("kernel: falling back to host compute", file=sys.stderr)
        y_list = []
        for e in range(N_EXPERTS):
            xg = x_pad[tok_pad[e]]
            a = xg @ W[e]
            b = xg @ V[e]
            yy = (a * (b / (1.0 + np.exp(-b)))) @ W_out[e]
            w_e = probs_pad[tok_pad[e], slot_pad[e]][:, None]
            y_list.append((yy * w_e).astype(np.float32))

    # ---- overflow tokens (host, fp32): the residual load imbalance ----
    if ov_tok:
        ot = np.asarray(ov_tok, dtype=np.int64)
        os_ = np.asarray(ov_slot, dtype=np.int64)
        oe = np.asarray(ov_expert, dtype=np.int64)
        y_ov = np.zeros((len(ot), D), np.float32)
        for e in np.unique(oe):
            m = oe == e
            xg = x_pad[ot[m]]
            a = xg @ W[e]
            b = xg @ V[e]
            y_ov[m] = (a * (b / (1.0 + np.exp(-b)))) @ W_out[e]
        y_ov *= probs_pad[ot, os_][:, None]
        y_list.append(y_ov)

    y_all = np.concatenate(y_list + [np.zeros((1, D), np.float32)], axis=0)
    # ---- combine (host): out[n] = sum_k y_scaled[pos_k(n)] ----
    n_rows = y_all.shape[0] - 1
    pos = np.where(pos_of_pair < 0, n_rows, pos_of_pair)
    out_flat = y_all[pos].reshape(N, TOP_K, D).sum(axis=1)
    return out_flat.reshape(Bq, Tq, D).astype(np.float32, copy=False)
